# revision 1
# baseline (speedup 1.0000x reference)
"""Bass/Trainium2 kernel for a 4-layer GraphSAGE GNN (mean aggregation).

Problem (hardcoded): N=100000 nodes, E=1200000 edges, x:[N,3] f32,
edge_index:[2,E] int64, hidden=64, out=2, log_softmax output.

  h1 = relu(mean_nbr(x) @ Wl1 + x @ Wr1 + b1)
  h2 = relu(mean_nbr(h1) @ Wl2 + h1 @ Wr2 + b2)
  h3 = relu(mean_nbr(h2) @ Wl3 + h2 @ Wr3 + b3)
  out = log_softmax(mean_nbr(h3) @ Wl4 + h3 @ Wr4 + b4)

Strategy (8 NeuronCores, node-partitioned):
- Core k owns nodes [k*12544, (k+1)*12544). Per layer each core aggregates
  its in-edges: dma_gather 256B rows from the (allgathered) full node table,
  dma_scatter_add into per-src-chunk HBM accumulators (4 chunks of 25088
  rows so gather indices fit in int16). Scatter calls are grouped into
  "waves" with unique dst indices per call (the SDMA CCE add loses updates
  when one call contains duplicate destinations); waves of the same chunk
  serialize on their accumulator, the 4 chunks run concurrently.
- Linear algebra per 512-node group: PE transposes agg to feature-major,
  3 PSUM-accumulated matmuls (Wl-part, Wr-part, bias via ones-rank-1),
  relu on ACT; mean division folded in as a per-partition scale before the
  transpose. h is kept feature-major in a DRAM ping-pong for the Wr matmul
  and node-major in the allgathered table for the next layer's gather.
- Layer 1 aggregates T1 = x @ Wl1 (so gather rows are 256B even though
  x rows are only 12B); layer 1's mean-part matmul is an identity.
"""

import os
import numpy as np
from contextlib import ExitStack

# debug/bench switches (unset in normal use)
_SKIP_EDGE = os.environ.get("K_SKIP_EDGE", "") != ""
_SKIP_CC = os.environ.get("K_SKIP_CC", "") != ""
_SKIP_EPI = os.environ.get("K_SKIP_EPI", "") != ""
_SCATTER_OFF = os.environ.get("K_SCATTER_OFF", "") != ""

# ---- problem constants (self-contained; do not read spec/reference) ----
N = 100000
E = 1200000
NCORES = 8
NPC = -(-N // (NCORES * 128)) * 128  # nodes per core = 12544 = 98 * 128
NPAD = NCORES * NPC         # 100352
NCHUNK = 4
CH = NPAD // NCHUNK         # 25088 gather-table rows per chunk (int16-safe)
F = 64
FIN = 3
FOUT = 2
SUB = 1024                  # edges per gather/scatter call (HW-safe descriptor
                            # count per SWDGE instruction; 2048+ crashes)
AGG_R = NPC + 128           # rows >= NPC are scatter-pad trash
GROUP = 512                 # nodes per epilogue group
NBLK = NPC // 128           # 98

_CACHE = {}


def _wrap_idx(idx: np.ndarray) -> np.ndarray:
    """Edge i -> idxs[i%16, i//16], replicated for the 8 Q7 cores."""
    w = idx.reshape(-1, 16).T.astype(np.int16)
    return np.tile(w, (8, 1))


def _occ_rank(d: np.ndarray):
    """Occurrence rank of each value of d (0 for first occurrence, ...)."""
    if len(d) == 0:
        return np.zeros(0, np.int64)
    order = np.argsort(d, kind="stable")
    ds = d[order]
    starts = np.r_[0, np.nonzero(np.diff(ds))[0] + 1]
    gid = np.cumsum(np.r_[0, (np.diff(ds) != 0).astype(np.int64)])
    rank_sorted = np.arange(len(ds)) - starts[gid]
    rank = np.empty(len(d), np.int64)
    rank[order] = rank_sorted
    return rank


def _pair_chunk(s_c: np.ndarray, d_c: np.ndarray):
    """Split a chunk's edges into paired-dst units and singles.

    Returns (pair_even_src, pair_odd_src, pair_rows, pair_waves,
             single_src, single_dst, single_waves) where waves are
    occurrence-rank arrays (unique row per wave guaranteed).
    """
    if len(d_c) == 0:
        z = np.zeros(0, np.int64)
        return z, z, z, [], z, z, []
    r = d_c // 2  # pair row
    lane = d_c & 1
    occ = _occ_rank(d_c)
    # m_r = min(count_even, count_odd) per pair row
    maxd = int(d_c.max()) + 1
    cnt = np.bincount(d_c, minlength=maxd + 2)
    c_even = cnt[0::2]
    c_odd = cnt[1::2]
    nrows = max(len(c_even), len(c_odd))
    c_even = np.pad(c_even, (0, nrows - len(c_even)))
    c_odd = np.pad(c_odd, (0, nrows - len(c_odd)))
    m = np.minimum(c_even, c_odd)  # paired depth per pair row
    paired_mask = occ < m[r]
    pe_mask = paired_mask & (lane == 0)
    po_mask = paired_mask & (lane == 1)
    # paired: for wave w, pair row r: even partner = w-th even edge of r,
    # odd partner = w-th odd edge of r. Sort by (wave, row) on each lane.
    def sort_wo(mask):
        ww = occ[mask]
        rr = r[mask]
        ss = s_c[mask]
        o = np.lexsort((rr, ww))
        return ss[o], rr[o], ww[o]

    pe_s, pe_r, pe_w = sort_wo(pe_mask)
    po_s, po_r, po_w = sort_wo(po_mask)
    assert np.array_equal(pe_r, po_r) and np.array_equal(pe_w, po_w)
    pair_wave_sizes = np.bincount(pe_w).tolist() if len(pe_w) else []

    sing = ~paired_mask
    s_occ = occ[sing] - m[r[sing]]  # rank among singles of this dst
    ss = s_c[sing]
    sd = d_c[sing]
    o = np.lexsort((sd, s_occ))
    ss, sd, sw = ss[o], sd[o], s_occ[o]
    single_wave_sizes = np.bincount(sw).tolist() if len(sw) else []
    return (pe_s, po_s, pe_r, pair_wave_sizes, ss, sd, single_wave_sizes)


def _preprocess(x: np.ndarray, edge_index: np.ndarray):
    src = np.asarray(edge_index[0], dtype=np.int64)
    dst = np.asarray(edge_index[1], dtype=np.int64)

    deg = np.bincount(dst, minlength=NPAD).astype(np.float32)
    invdeg = 1.0 / np.maximum(deg, 1.0)

    owner = dst // NPC
    # per (core, chunk): paired/single streams
    per = [[None] * NCHUNK for _ in range(NCORES)]
    for k in range(NCORES):
        m = owner == k
        s_k = src[m]
        d_k = dst[m] - k * NPC
        c_k = s_k // CH
        for c in range(NCHUNK):
            mc = c_k == c
            per[k][c] = _pair_chunk(s_k[mc] - c * CH, d_k[mc])

    # shared (max-over-cores) wave structure: pair waves in PAIR UNITS
    # (128-aligned), single waves in edges (128-aligned)
    PWV, SWV = [], []
    for c in range(NCHUNK):
        npw = max(len(per[k][c][3]) for k in range(NCORES))
        pw = []
        for w in range(npw):
            mx = max(
                (per[k][c][3][w] if w < len(per[k][c][3]) else 0)
                for k in range(NCORES)
            )
            pw.append(int(np.ceil(mx / 128) * 128))
        PWV.append(pw)
        nsw = max(len(per[k][c][6]) for k in range(NCORES))
        sw = []
        for w in range(nsw):
            mx = max(
                (per[k][c][6][w] if w < len(per[k][c][6]) else 0)
                for k in range(NCORES)
            )
            sw.append(int(np.ceil(mx / 128) * 128))
        SWV.append(sw)
    NPU = [sum(pw) for pw in PWV]      # pair units per chunk (shared)
    NSE = [sum(sw) for sw in SWV]      # single edges per chunk (shared)
    P = [2 * NPU[c] + NSE[c] for c in range(NCHUNK)]  # edges per chunk

    TRASH_PAIR = NPC // 2  # pair rows >= this are trash (64 rows)

    # build padded streams per core
    gidx_maps, sidx_maps = [], []
    for k in range(NCORES):
        gparts, spair_parts, ssing_parts = [], [], []
        for c in range(NCHUNK):
            pe_s, po_s, pe_r, pwsz, ss, sd, swsz = per[k][c]
            # paired region: per shared wave, pad to PWV[c][w] units
            ev, od, rw = [], [], []
            off = 0
            for w, wcap in enumerate(PWV[c]):
                n = pwsz[w] if w < len(pwsz) else 0
                ev.append(pe_s[off : off + n])
                od.append(po_s[off : off + n])
                rw.append(pe_r[off : off + n])
                off += n
                padn = wcap - n
                ev.append(np.zeros(padn, np.int64))
                od.append(np.zeros(padn, np.int64))
                rw.append(TRASH_PAIR + (np.arange(padn) % 64))
            ev = np.concatenate(ev) if ev else np.zeros(0, np.int64)
            od = np.concatenate(od) if od else np.zeros(0, np.int64)
            rw = np.concatenate(rw) if rw else np.zeros(0, np.int64)
            assert len(ev) == NPU[c]
            # gather stream for paired region: blocks of 128 units ->
            # [128 even][128 odd]
            if NPU[c]:
                blocks = ev.reshape(-1, 128), od.reshape(-1, 128)
                inter = np.stack(blocks, axis=1).reshape(-1)  # [nb,2,128]->flat
                gparts.append(inter)
            spair_parts.append(rw)
            # singles region
            sv, dv = [], []
            off = 0
            for w, wcap in enumerate(SWV[c]):
                n = swsz[w] if w < len(swsz) else 0
                sv.append(ss[off : off + n])
                dv.append(sd[off : off + n])
                off += n
                padn = wcap - n
                sv.append(np.zeros(padn, np.int64))
                dv.append(NPC + (np.arange(padn) % 128))
            sv = np.concatenate(sv) if sv else np.zeros(0, np.int64)
            dv = np.concatenate(dv) if dv else np.zeros(0, np.int64)
            assert len(sv) == NSE[c]
            gparts.append(sv)
            ssing_parts.append(dv)
        gidx_maps.append(_wrap_idx(np.concatenate(gparts)))
        sidx_maps.append(
            _wrap_idx(np.concatenate(spair_parts + ssing_parts))
        )

    # per-core transposed features and inverse degree in [p, blk] layout
    xpad = np.zeros((NPAD, FIN), np.float32)
    xpad[:N] = x
    xT = [np.ascontiguousarray(xpad[k * NPC : (k + 1) * NPC].T) for k in range(NCORES)]
    inv_pb = [
        np.ascontiguousarray(
            invdeg[k * NPC : (k + 1) * NPC].reshape(NBLK, 128).T
        )
        for k in range(NCORES)
    ]
    meta = dict(PWV=PWV, SWV=SWV, NPU=NPU, NSE=NSE, P=P)
    return meta, gidx_maps, sidx_maps, xT, inv_pb


def _build_module(meta):
    import concourse.bass as bass
    import concourse.bacc as bacc
    import concourse.mybir as mybir
    from concourse import tile
    from concourse import library_config
    from concourse import masks

    f32 = mybir.dt.float32
    i16 = mybir.dt.int16
    AF = mybir.ActivationFunctionType
    ALU = mybir.AluOpType

    PWV, SWV = meta["PWV"], meta["SWV"]
    NPU, NSE, P = meta["NPU"], meta["NSE"], meta["P"]
    LG = sum(P) // 16
    LS = (sum(NPU) + sum(NSE)) // 16
    nc = bacc.Bacc(None, target_bir_lowering=False)

    # ---- parameters ----
    xT_p = nc.declare_dram_parameter("xT", [FIN, NPC], f32, isOutput=False)
    gidx_p = nc.declare_dram_parameter("gidx", [128, LG], i16, isOutput=False)
    sidx_p = nc.declare_dram_parameter("sidx", [128, LS], i16, isOutput=False)
    inv_p = nc.declare_dram_parameter("invdeg", [128, NBLK], f32, isOutput=False)
    wl_p, wr_p, b_p = [None], [None], [None]
    for l in range(1, 5):
        din = FIN if l == 1 else F
        dout = FOUT if l == 4 else F
        wl_p.append(nc.declare_dram_parameter(f"Wl{l}", [din, dout], f32, isOutput=False))
        wr_p.append(nc.declare_dram_parameter(f"Wr{l}", [din, dout], f32, isOutput=False))
        b_p.append(nc.declare_dram_parameter(f"b{l}", [1, dout], f32, isOutput=False))
    out_p = nc.declare_dram_parameter("out_shard", [NPC, FOUT], f32, isOutput=True)

    # ---- internal DRAM ----
    T = [None] + [
        nc.dram_tensor(f"T{l}", [NPAD, F], f32, addr_space="Shared") for l in range(1, 5)
    ]
    sh = [None] + [nc.dram_tensor(f"sh{l}", [NPC, F], f32) for l in range(1, 5)]
    agg = [
        [nc.dram_tensor(f"agg{l}_{c}", [AGG_R, F], f32) for c in range(NCHUNK)]
        for l in range(1, 5)
    ]
    hTd = [nc.dram_tensor(f"hT{i}", [F, NPC], f32) for i in range(2)]  # ping-pong

    # groups: (start_block, n_blocks)
    groups = []
    b0 = 0
    while b0 < NBLK:
        nb = min(GROUP // 128, NBLK - b0)
        groups.append((b0, nb))
        b0 += nb

    # chunk edge-call layout: per chunk, per 1024-edge gather tile:
    # (edge_off, n_edges, pair_slices, single_slices)
    # pair_slices: (rel_unit, n_units, abs_unit); single: (rel_e, n_e, abs_e)
    def chunk_calls(c):
        pbounds, u = [], 0
        for wn in PWV[c]:
            pbounds.append((u, u + wn))
            u += wn
        sbounds, e = [], 2 * NPU[c]
        for wn in SWV[c]:
            sbounds.append((e, e + wn))
            e += wn
        calls = []
        off = 0
        while off < P[c]:
            n = min(SUB, P[c] - off)
            pe = min(off + n, 2 * NPU[c])
            pslices = []
            if off < 2 * NPU[c]:
                tu0, tu1 = off // 2, pe // 2
                for wlo, whi in pbounds:
                    lo, hi = max(wlo, tu0), min(whi, tu1)
                    if lo < hi:
                        pslices.append((lo - tu0, hi - lo, lo))
                    if wlo >= tu1:
                        break
            sslices = []
            if off + n > 2 * NPU[c]:
                for wlo, whi in sbounds:
                    lo, hi = max(wlo, off), min(whi, off + n)
                    if lo < hi:
                        sslices.append((lo - off, hi - lo, lo - 2 * NPU[c]))
                    if wlo >= off + n:
                        break
            calls.append((off, n, pslices, sslices))
            off += n
        return calls

    chunk_base = [sum(P[:c]) for c in range(NCHUNK)]
    pair_base = [sum(NPU[:c]) for c in range(NCHUNK)]
    sing_base = [sum(NPU) + sum(NSE[:c]) for c in range(NCHUNK)]

    with tile.TileContext(nc) as tc, ExitStack() as ctx:
        idxp = ctx.enter_context(tc.tile_pool(name="idx", bufs=1))
        constp = ctx.enter_context(tc.tile_pool(name="const", bufs=1))
        zerop = ctx.enter_context(tc.tile_pool(name="zero", bufs=1))
        edgep = ctx.enter_context(tc.tile_pool(name="edge", bufs=4))
        grpp = ctx.enter_context(tc.tile_pool(name="grp", bufs=3))
        psA = ctx.enter_context(tc.tile_pool(name="psA", bufs=2, space="PSUM"))
        psB = ctx.enter_context(tc.tile_pool(name="psB", bufs=2, space="PSUM"))
        psC = ctx.enter_context(tc.tile_pool(name="psC", bufs=2, space="PSUM"))

        nc.gpsimd.load_library(library_config.mlp)

        # ---- persistent constants ----
        gi = idxp.tile([128, LG], i16)
        si = idxp.tile([128, LS], i16)
        nc.sync.dma_start(gi[:], gidx_p[:])
        nc.sync.dma_start(si[:], sidx_p[:])

        inv = constp.tile([128, NBLK], f32)
        nc.sync.dma_start(inv[:], inv_p[:])
        ident = constp.tile([128, 128], f32)
        masks.make_identity(nc, ident[:])
        ones = constp.tile([1, GROUP], f32)
        nc.vector.memset(ones[:], 1.0)

        wl_t, wr_t, b_t = [None], [None], [None]
        for l in range(1, 5):
            din = FIN if l == 1 else F
            dout = FOUT if l == 4 else F
            t1 = constp.tile([din, dout], f32, tag=f"wl{l}")
            t2 = constp.tile([din, dout], f32, tag=f"wr{l}")
            t3 = constp.tile([1, dout], f32, tag=f"b{l}")
            nc.sync.dma_start(t1[:], wl_p[l][:])
            nc.sync.dma_start(t2[:], wr_p[l][:])
            nc.sync.dma_start(t3[:], b_p[l][:])
            wl_t.append(t1)
            wr_t.append(t2)
            b_t.append(t3)

        zt = zerop.tile([128, (AGG_R // 128) * F], f32)
        nc.vector.memset(zt[:], 0.0)

        _REPS = int(os.environ.get("K_REPS", "1"))
        for _rep in range(_REPS):
            # ---- layer-1 table: T1 = x @ Wl1, node-major, then allgather ----
            for g0, nb in groups:
                xt = grpp.tile([FIN, GROUP], f32, tag="prevT")
                nc.sync.dma_start(
                    xt[:, : nb * 128], xT_p[:, g0 * 128 : (g0 + nb) * 128]
                )
                pnm = psC.tile([128, GROUP // 128, F], f32, tag="nm")
                for a in range(nb):
                    nc.tensor.matmul(
                        pnm[:, a, :],
                        xt[:, a * 128 : (a + 1) * 128],
                        wl_t[1][:],
                        start=True,
                        stop=True,
                    )
                hnm = grpp.tile([128, GROUP // 128, F], f32, tag="hnm")
                nc.scalar.activation(hnm[:, :nb, :], pnm[:, :nb, :], AF.Copy)
                nc.sync.dma_start(
                    sh[1][g0 * 128 : (g0 + nb) * 128, :].rearrange(
                        "(a p) f -> p a f", p=128
                    ),
                    hnm[:, :nb, :],
                )
            if not _SKIP_CC:
                nc.gpsimd.collective_compute(
                    "AllGather",
                    mybir.AluOpType.bypass,
                    replica_groups=[list(range(NCORES))],
                    ins=[sh[1][:]],
                    outs=[T[1][:]],
                )

            # ---- layers ----
            for l in range(1, 5):
                din = FIN if l == 1 else F
                dout = FOUT if l == 4 else F
                prev_dram = xT_p if l == 1 else hTd[l % 2]
                next_hT = hTd[(l + 1) % 2]

                # zero the 4 chunk accumulators
                for c in range(NCHUNK):
                    nc.sync.dma_start(
                        agg[l - 1][c].rearrange("(p a) f -> p (a f)", p=128), zt[:]
                    )

                # edge phase: gather + paired/single wave scatters, chunks
                # interleaved so the four wave-chains overlap
                percall = [chunk_calls(c) for c in range(NCHUNK)]
                maxcalls = max(len(p) for p in percall)
                for i in range(maxcalls):
                    for c in range(NCHUNK):
                        if _SKIP_EDGE or i >= len(percall[c]):
                            continue
                        off, n, pslices, sslices = percall[c][i]
                        tbl = T[l][c * CH : (c + 1) * CH, :]
                        gbase = chunk_base[c] + off
                        gt = edgep.tile([128, SUB // 128, F], f32, tag=f"e{c}")
                        nc.gpsimd.dma_gather(
                            gt[:, : n // 128, :],
                            tbl,
                            gi[:, gbase // 16 : (gbase + n) // 16],
                            n,
                            n,
                            F,
                        )
                        if _SCATTER_OFF:
                            continue
                        gtp = gt[:].rearrange(
                            "p (a two) f -> p a (two f)", two=2
                        )
                        aggp = agg[l - 1][c].rearrange(
                            "(a two) f -> a (two f)", two=2
                        )
                        for rel, cnt, au in pslices:
                            sb = pair_base[c] + au
                            nc.gpsimd.dma_scatter_add(
                                aggp,
                                gtp[:, rel // 128 : (rel + cnt) // 128, :],
                                si[:, sb // 16 : (sb + cnt) // 16],
                                cnt,
                                cnt,
                                2 * F,
                            )
                        for rel, cnt, ae in sslices:
                            sb = sing_base[c] + ae
                            nc.gpsimd.dma_scatter_add(
                                agg[l - 1][c][:],
                                gt[:, rel // 128 : (rel + cnt) // 128, :],
                                si[:, sb // 16 : (sb + cnt) // 16],
                                cnt,
                                cnt,
                                F,
                            )

                # epilogue per 512-node group
                for gidx_g, (g0, nb) in enumerate(groups):
                    if _SKIP_EPI and l < 4:
                        continue
                    rows = slice(g0 * 128, (g0 + nb) * 128)
                    asb = grpp.tile([128, GROUP // 128, F], f32, tag="agg")
                    nc.sync.dma_start(
                        asb[:, :nb, :],
                        agg[l - 1][0][rows, :].rearrange("(a p) f -> p a f", p=128),
                    )
                    for c in range(1, NCHUNK):
                        a2 = grpp.tile([128, GROUP // 128, F], f32, tag=f"agg{c}")
                        nc.sync.dma_start(
                            a2[:, :nb, :],
                            agg[l - 1][c][rows, :].rearrange("(a p) f -> p a f", p=128),
                        )
                        nc.vector.tensor_tensor(
                            asb[:, :nb, :], asb[:, :nb, :], a2[:, :nb, :], ALU.add
                        )
                    # mean scale (per-partition scalar per block)
                    for a in range(nb):
                        nc.vector.tensor_scalar(
                            asb[:, a, :],
                            asb[:, a, :],
                            inv[:, g0 + a : g0 + a + 1],
                            None,
                            ALU.mult,
                        )
                    # transpose to feature-major
                    pmt = psA.tile([F, GROUP], f32, tag="mt")
                    for a in range(nb):
                        nc.tensor.transpose(
                            pmt[:, a * 128 : (a + 1) * 128], asb[:, a, :], ident[:]
                        )
                    mt = grpp.tile([F, GROUP], f32, tag="mt_sb")
                    nc.vector.tensor_copy(mt[:, : nb * 128], pmt[:, : nb * 128])

                    # prev features (feature-major) for the Wr part
                    pv = grpp.tile([din, GROUP], f32, tag="prevT")
                    nc.sync.dma_start(pv[:, : nb * 128], prev_dram[:, rows])

                    ph = psB.tile([dout, GROUP], f32, tag="h")
                    if l == 1:
                        nc.tensor.matmul(
                            ph[:, : nb * 128],
                            ident[:F, :F],
                            mt[:, : nb * 128],
                            start=True,
                            stop=False,
                        )
                    else:
                        nc.tensor.matmul(
                            ph[:, : nb * 128],
                            wl_t[l][:],
                            mt[:, : nb * 128],
                            start=True,
                            stop=False,
                        )
                    nc.tensor.matmul(
                        ph[:, : nb * 128],
                        wr_t[l][:],
                        pv[:, : nb * 128],
                        start=False,
                        stop=False,
                    )
                    nc.tensor.matmul(
                        ph[:, : nb * 128],
                        b_t[l][:],
                        ones[:, : nb * 128],
                        start=False,
                        stop=True,
                    )

                    if l < 4:
                        hT_sb = grpp.tile([F, GROUP], f32, tag="hT_sb")
                        nc.scalar.activation(
                            hT_sb[:, : nb * 128], ph[:, : nb * 128], AF.Relu
                        )
                        nc.sync.dma_start(next_hT[:, rows], hT_sb[:, : nb * 128])
                        # node-major for the next table
                        pnm = psC.tile([128, GROUP // 128, F], f32, tag="nm")
                        for a in range(nb):
                            nc.tensor.transpose(
                                pnm[:, a, :],
                                hT_sb[:, a * 128 : (a + 1) * 128],
                                ident[:F, :F],
                            )
                        hnm = grpp.tile([128, GROUP // 128, F], f32, tag="hnm")
                        nc.vector.tensor_copy(hnm[:, :nb, :], pnm[:, :nb, :])
                        nc.sync.dma_start(
                            sh[l + 1][rows, :].rearrange("(a p) f -> p a f", p=128),
                            hnm[:, :nb, :],
                        )
                    else:
                        # logits -> node-major -> log_softmax -> out_shard
                        zsb = grpp.tile([FOUT, GROUP], f32, tag="zsb")
                        nc.vector.tensor_copy(zsb[:, : nb * 128], ph[:, : nb * 128])
                        pz = psC.tile([128, GROUP // 128, FOUT], f32, tag="znm")
                        for a in range(nb):
                            nc.tensor.transpose(
                                pz[:, a, :],
                                zsb[:, a * 128 : (a + 1) * 128],
                                ident[:FOUT, :FOUT],
                            )
                        z = grpp.tile([128, GROUP // 128, FOUT], f32, tag="z")
                        nc.vector.tensor_copy(z[:, :nb, :], pz[:, :nb, :])
                        z0 = z[:, :nb, 0:1]
                        z1 = z[:, :nb, 1:2]
                        m = grpp.tile([128, GROUP // 128, 1], f32, tag="m")
                        nc.vector.tensor_tensor(m[:, :nb, :], z0, z1, ALU.max)
                        d = grpp.tile([128, GROUP // 128, FOUT], f32, tag="d")
                        nc.vector.tensor_tensor(d[:, :nb, 0:1], z0, m[:, :nb, :], ALU.subtract)
                        nc.vector.tensor_tensor(d[:, :nb, 1:2], z1, m[:, :nb, :], ALU.subtract)
                        e = grpp.tile([128, GROUP // 128, FOUT], f32, tag="e")
                        nc.scalar.activation(e[:, :nb, :], d[:, :nb, :], AF.Exp)
                        s = grpp.tile([128, GROUP // 128, 1], f32, tag="s")
                        nc.vector.tensor_tensor(
                            s[:, :nb, :], e[:, :nb, 0:1], e[:, :nb, 1:2], ALU.add
                        )
                        ls = grpp.tile([128, GROUP // 128, 1], f32, tag="ls")
                        nc.scalar.activation(ls[:, :nb, :], s[:, :nb, :], AF.Ln)
                        o = grpp.tile([128, GROUP // 128, FOUT], f32, tag="o")
                        nc.vector.tensor_tensor(
                            o[:, :nb, 0:1], d[:, :nb, 0:1], ls[:, :nb, :], ALU.subtract
                        )
                        nc.vector.tensor_tensor(
                            o[:, :nb, 1:2], d[:, :nb, 1:2], ls[:, :nb, :], ALU.subtract
                        )
                        nc.sync.dma_start(
                            out_p[rows, :].rearrange("(a p) f -> p a f", p=128),
                            o[:, :nb, :],
                        )

                if l < 4 and not (_SKIP_CC or _SKIP_EPI):
                    nc.gpsimd.collective_compute(
                        "AllGather",
                        mybir.AluOpType.bypass,
                        replica_groups=[list(range(NCORES))],
                        ins=[sh[l + 1][:]],
                        outs=[T[l + 1][:]],
                    )

    nc.compile()
    return nc


def _prepare(inputs):
    x = np.asarray(inputs["x"], dtype=np.float32)
    edge_index = np.asarray(inputs["edge_index"])
    meta, gidx_maps, sidx_maps, xT, inv_pb = _preprocess(x, edge_index)

    key = (
        tuple(meta["P"]),
        tuple(meta["NPU"]),
        tuple(meta["NSE"]),
        _SKIP_EDGE,
        _SKIP_CC,
        _SKIP_EPI,
        _SCATTER_OFF,
        os.environ.get("K_REPS", "1"),
    )
    if key not in _CACHE:
        _CACHE[key] = _build_module(meta)
    nc = _CACHE[key]

    in_maps = []
    for k in range(NCORES):
        m = {
            "xT": xT[k],
            "gidx": gidx_maps[k],
            "sidx": sidx_maps[k],
            "invdeg": inv_pb[k],
        }
        for l in range(1, 5):
            m[f"Wl{l}"] = np.asarray(inputs[f"Wl{l}"], np.float32)
            m[f"Wr{l}"] = np.asarray(inputs[f"Wr{l}"], np.float32)
            m[f"b{l}"] = np.asarray(inputs[f"b{l}"], np.float32).reshape(1, -1)
        in_maps.append(m)
    return nc, in_maps


def _run(inputs, trace=False):
    from concourse.bass_utils import run_bass_kernel_spmd

    nc, in_maps = _prepare(inputs)
    r = run_bass_kernel_spmd(nc, in_maps, list(range(NCORES)), trace=trace)
    out = np.concatenate(
        [r.results[k]["out_shard"] for k in range(NCORES)], axis=0
    )[:N]
    return out.astype(np.float32), r


def kernel(**inputs) -> np.ndarray:
    out, _ = _run(inputs)
    return out



# revision 5
# speedup vs baseline: 1.6246x; 1.6246x over previous
"""Bass/Trainium2 kernel for a 4-layer GraphSAGE GNN (mean aggregation).

Problem (hardcoded): N=100000 nodes, E=1200000 edges, x:[N,3] f32,
edge_index:[2,E] int64, hidden=64, out=2, log_softmax output.

  h1 = relu(mean_nbr(x) @ Wl1 + x @ Wr1 + b1)
  h2 = relu(mean_nbr(h1) @ Wl2 + h1 @ Wr2 + b2)
  h3 = relu(mean_nbr(h2) @ Wl3 + h2 @ Wr3 + b3)
  out = log_softmax(mean_nbr(h3) @ Wl4 + h3 @ Wr4 + b4)

Strategy (8 NeuronCores, node-partitioned):
- Core k owns nodes [k*12544, (k+1)*12544). Per layer each core aggregates
  its in-edges: dma_gather 256B rows from the (allgathered) full node table,
  dma_scatter_add into per-src-chunk HBM accumulators (4 chunks of 25088
  rows so gather indices fit in int16). Scatter calls are grouped into
  "waves" with unique dst indices per call (the SDMA CCE add loses updates
  when one call contains duplicate destinations); waves of the same chunk
  serialize on their accumulator, the 4 chunks run concurrently.
- Linear algebra per 512-node group: PE transposes agg to feature-major,
  3 PSUM-accumulated matmuls (Wl-part, Wr-part, bias via ones-rank-1),
  relu on ACT; mean division folded in as a per-partition scale before the
  transpose. h is kept feature-major in a DRAM ping-pong for the Wr matmul
  and node-major in the allgathered table for the next layer's gather.
- Layer 1 aggregates T1 = x @ Wl1 (so gather rows are 256B even though
  x rows are only 12B); layer 1's mean-part matmul is an identity.
"""

import os
import numpy as np
from contextlib import ExitStack

# debug/bench switches (unset in normal use)
_SKIP_EDGE = os.environ.get("K_SKIP_EDGE", "") != ""
_SKIP_CC = os.environ.get("K_SKIP_CC", "") != ""
_SKIP_EPI = os.environ.get("K_SKIP_EPI", "") != ""
_SCATTER_OFF = os.environ.get("K_SCATTER_OFF", "") != ""

# ---- problem constants (self-contained; do not read spec/reference) ----
N = 100000
E = 1200000
NCORES = 8
NPC = -(-N // (NCORES * 128)) * 128  # nodes per core = 12544 = 98 * 128
NPAD = NCORES * NPC         # 100352
NCHUNK = 4
CH = NPAD // NCHUNK         # 25088 gather-table rows per chunk (int16-safe)
F = 64
FIN = 3
FOUT = 2
SUB = 1024                  # edges per gather/scatter call (HW-safe descriptor
                            # count per SWDGE instruction; 2048+ crashes)
AGG_R = NPC + 128           # rows >= NPC are scatter-pad trash
GROUP = 512                 # nodes per epilogue group
NBLK = NPC // 128           # 98

_CACHE = {}


def _wrap_idx(idx: np.ndarray) -> np.ndarray:
    """Edge i -> idxs[i%16, i//16], replicated for the 8 Q7 cores."""
    w = idx.reshape(-1, 16).T.astype(np.int16)
    return np.tile(w, (8, 1))


def _occ_rank(d: np.ndarray):
    """Occurrence rank of each value of d (0 for first occurrence, ...)."""
    if len(d) == 0:
        return np.zeros(0, np.int64)
    order = np.argsort(d, kind="stable")
    ds = d[order]
    starts = np.r_[0, np.nonzero(np.diff(ds))[0] + 1]
    gid = np.cumsum(np.r_[0, (np.diff(ds) != 0).astype(np.int64)])
    rank_sorted = np.arange(len(ds)) - starts[gid]
    rank = np.empty(len(d), np.int64)
    rank[order] = rank_sorted
    return rank


def _pair_chunk(s_c: np.ndarray, d_c: np.ndarray):
    """Split a chunk's edges into paired-dst units and singles.

    Returns (pair_even_src, pair_odd_src, pair_rows, pair_waves,
             single_src, single_dst, single_waves) where waves are
    occurrence-rank arrays (unique row per wave guaranteed).
    """
    if len(d_c) == 0:
        z = np.zeros(0, np.int64)
        return z, z, z, [], z, z, []
    r = d_c // 2  # pair row
    lane = d_c & 1
    occ = _occ_rank(d_c)
    # m_r = min(count_even, count_odd) per pair row
    maxd = int(d_c.max()) + 1
    cnt = np.bincount(d_c, minlength=maxd + 2)
    c_even = cnt[0::2]
    c_odd = cnt[1::2]
    nrows = max(len(c_even), len(c_odd))
    c_even = np.pad(c_even, (0, nrows - len(c_even)))
    c_odd = np.pad(c_odd, (0, nrows - len(c_odd)))
    m = np.minimum(c_even, c_odd)  # paired depth per pair row
    paired_mask = occ < m[r]
    pe_mask = paired_mask & (lane == 0)
    po_mask = paired_mask & (lane == 1)
    # paired: for wave w, pair row r: even partner = w-th even edge of r,
    # odd partner = w-th odd edge of r. Sort by (wave, row) on each lane.
    def sort_wo(mask):
        ww = occ[mask]
        rr = r[mask]
        ss = s_c[mask]
        o = np.lexsort((rr, ww))
        return ss[o], rr[o], ww[o]

    pe_s, pe_r, pe_w = sort_wo(pe_mask)
    po_s, po_r, po_w = sort_wo(po_mask)
    assert np.array_equal(pe_r, po_r) and np.array_equal(pe_w, po_w)
    pair_wave_sizes = np.bincount(pe_w).tolist() if len(pe_w) else []

    sing = ~paired_mask
    s_occ = occ[sing] - m[r[sing]]  # rank among singles of this dst
    ss = s_c[sing]
    sd = d_c[sing]
    o = np.lexsort((sd, s_occ))
    ss, sd, sw = ss[o], sd[o], s_occ[o]
    single_wave_sizes = np.bincount(sw).tolist() if len(sw) else []
    return (pe_s, po_s, pe_r, pair_wave_sizes, ss, sd, single_wave_sizes)


def _preprocess(x: np.ndarray, edge_index: np.ndarray):
    src = np.asarray(edge_index[0], dtype=np.int64)
    dst = np.asarray(edge_index[1], dtype=np.int64)

    deg = np.bincount(dst, minlength=NPAD).astype(np.float32)
    invdeg = 1.0 / np.maximum(deg, 1.0)

    owner = dst // NPC
    # per (core, chunk): paired/single streams
    per = [[None] * NCHUNK for _ in range(NCORES)]
    for k in range(NCORES):
        m = owner == k
        s_k = src[m]
        d_k = dst[m] - k * NPC
        c_k = s_k // CH
        for c in range(NCHUNK):
            mc = c_k == c
            per[k][c] = _pair_chunk(s_k[mc] - c * CH, d_k[mc])

    # shared (max-over-cores) wave structure: pair waves in PAIR UNITS
    # (128-aligned), single waves in edges (128-aligned)
    PWV, SWV = [], []
    for c in range(NCHUNK):
        npw = max(len(per[k][c][3]) for k in range(NCORES))
        pw = []
        for w in range(npw):
            mx = max(
                (per[k][c][3][w] if w < len(per[k][c][3]) else 0)
                for k in range(NCORES)
            )
            pw.append(int(np.ceil(mx / 128) * 128))
        PWV.append(pw)
        nsw = max(len(per[k][c][6]) for k in range(NCORES))
        sw = []
        for w in range(nsw):
            mx = max(
                (per[k][c][6][w] if w < len(per[k][c][6]) else 0)
                for k in range(NCORES)
            )
            sw.append(int(np.ceil(mx / 128) * 128))
        SWV.append(sw)
    NPU = [sum(pw) for pw in PWV]      # pair units per chunk (shared)
    NSE = [sum(sw) for sw in SWV]      # single edges per chunk (shared)
    P = [2 * NPU[c] + NSE[c] for c in range(NCHUNK)]  # edges per chunk

    TRASH_PAIR = NPC // 2  # pair rows >= this are trash (64 rows)

    # build padded streams per core
    gidx_maps, sidx_maps = [], []
    for k in range(NCORES):
        gparts, spair_parts, ssing_parts = [], [], []
        for c in range(NCHUNK):
            pe_s, po_s, pe_r, pwsz, ss, sd, swsz = per[k][c]
            # paired region: per shared wave, pad to PWV[c][w] units
            ev, od, rw = [], [], []
            off = 0
            for w, wcap in enumerate(PWV[c]):
                n = pwsz[w] if w < len(pwsz) else 0
                ev.append(pe_s[off : off + n])
                od.append(po_s[off : off + n])
                rw.append(pe_r[off : off + n])
                off += n
                padn = wcap - n
                ev.append(np.zeros(padn, np.int64))
                od.append(np.zeros(padn, np.int64))
                rw.append(TRASH_PAIR + (np.arange(padn) % 64))
            ev = np.concatenate(ev) if ev else np.zeros(0, np.int64)
            od = np.concatenate(od) if od else np.zeros(0, np.int64)
            rw = np.concatenate(rw) if rw else np.zeros(0, np.int64)
            assert len(ev) == NPU[c]
            # gather stream for paired region: blocks of 128 units ->
            # [128 even][128 odd]
            if NPU[c]:
                blocks = ev.reshape(-1, 128), od.reshape(-1, 128)
                inter = np.stack(blocks, axis=1).reshape(-1)  # [nb,2,128]->flat
                gparts.append(inter)
            spair_parts.append(rw)
            # singles region
            sv, dv = [], []
            off = 0
            for w, wcap in enumerate(SWV[c]):
                n = swsz[w] if w < len(swsz) else 0
                sv.append(ss[off : off + n])
                dv.append(sd[off : off + n])
                off += n
                padn = wcap - n
                sv.append(np.zeros(padn, np.int64))
                dv.append(NPC + (np.arange(padn) % 128))
            sv = np.concatenate(sv) if sv else np.zeros(0, np.int64)
            dv = np.concatenate(dv) if dv else np.zeros(0, np.int64)
            assert len(sv) == NSE[c]
            gparts.append(sv)
            ssing_parts.append(dv)
        gidx_maps.append(_wrap_idx(np.concatenate(gparts)))
        sidx_maps.append(
            _wrap_idx(np.concatenate(spair_parts + ssing_parts))
        )

    # per-core transposed features and inverse degree in [p, blk] layout
    xpad = np.zeros((NPAD, FIN), np.float32)
    xpad[:N] = x
    xT = [np.ascontiguousarray(xpad[k * NPC : (k + 1) * NPC].T) for k in range(NCORES)]
    inv_pb = [
        np.ascontiguousarray(
            invdeg[k * NPC : (k + 1) * NPC].reshape(NBLK, 128).T
        )
        for k in range(NCORES)
    ]
    meta = dict(PWV=PWV, SWV=SWV, NPU=NPU, NSE=NSE, P=P)
    return meta, gidx_maps, sidx_maps, xT, inv_pb


def _build_module(meta):
    import concourse.bass as bass
    import concourse.bacc as bacc
    import concourse.mybir as mybir
    from concourse import tile
    from concourse import library_config
    from concourse import masks

    f32 = mybir.dt.float32
    i16 = mybir.dt.int16
    AF = mybir.ActivationFunctionType
    ALU = mybir.AluOpType

    PWV, SWV = meta["PWV"], meta["SWV"]
    NPU, NSE, P = meta["NPU"], meta["NSE"], meta["P"]
    LG = sum(P) // 16
    LS = (sum(NPU) + sum(NSE)) // 16
    nc = bacc.Bacc(None, target_bir_lowering=False, num_swdge_queues=4)

    # ---- parameters ----
    xT_p = nc.declare_dram_parameter("xT", [FIN, NPC], f32, isOutput=False)
    gidx_p = nc.declare_dram_parameter("gidx", [128, LG], i16, isOutput=False)
    sidx_p = nc.declare_dram_parameter("sidx", [128, LS], i16, isOutput=False)
    inv_p = nc.declare_dram_parameter("invdeg", [128, NBLK], f32, isOutput=False)
    wl_p, wr_p, b_p = [None], [None], [None]
    for l in range(1, 5):
        din = FIN if l == 1 else F
        dout = FOUT if l == 4 else F
        wl_p.append(nc.declare_dram_parameter(f"Wl{l}", [din, dout], f32, isOutput=False))
        wr_p.append(nc.declare_dram_parameter(f"Wr{l}", [din, dout], f32, isOutput=False))
        b_p.append(nc.declare_dram_parameter(f"b{l}", [1, dout], f32, isOutput=False))
    out_p = nc.declare_dram_parameter("out_shard", [NPC, FOUT], f32, isOutput=True)

    # ---- internal DRAM ----
    T = [None] + [
        nc.dram_tensor(f"T{l}", [NPAD, F], f32, addr_space="Shared") for l in range(1, 5)
    ]
    sh = [None] + [nc.dram_tensor(f"sh{l}", [NPC, F], f32) for l in range(1, 5)]
    agg = [
        [nc.dram_tensor(f"agg{l}_{c}", [AGG_R, F], f32) for c in range(NCHUNK)]
        for l in range(1, 5)
    ]
    hTd = [nc.dram_tensor(f"hT{i}", [F, NPC], f32) for i in range(2)]  # ping-pong

    # groups: (start_block, n_blocks)
    groups = []
    b0 = 0
    while b0 < NBLK:
        nb = min(GROUP // 128, NBLK - b0)
        groups.append((b0, nb))
        b0 += nb

    # chunk edge-call layout: per chunk, per 1024-edge gather tile:
    # (edge_off, n_edges, pair_slices, single_slices)
    # pair_slices: (rel_unit, n_units, abs_unit); single: (rel_e, n_e, abs_e)
    def chunk_calls(c):
        pbounds, u = [], 0
        for wn in PWV[c]:
            pbounds.append((u, u + wn))
            u += wn
        sbounds, e = [], 2 * NPU[c]
        for wn in SWV[c]:
            sbounds.append((e, e + wn))
            e += wn
        calls = []
        off = 0
        while off < P[c]:
            n = min(SUB, P[c] - off)
            pe = min(off + n, 2 * NPU[c])
            pslices = []
            if off < 2 * NPU[c]:
                tu0, tu1 = off // 2, pe // 2
                for wlo, whi in pbounds:
                    lo, hi = max(wlo, tu0), min(whi, tu1)
                    if lo < hi:
                        pslices.append((lo - tu0, hi - lo, lo))
                    if wlo >= tu1:
                        break
            sslices = []
            if off + n > 2 * NPU[c]:
                for wlo, whi in sbounds:
                    lo, hi = max(wlo, off), min(whi, off + n)
                    if lo < hi:
                        sslices.append((lo - off, hi - lo, lo - 2 * NPU[c]))
                    if wlo >= off + n:
                        break
            calls.append((off, n, pslices, sslices))
            off += n
        return calls

    chunk_base = [sum(P[:c]) for c in range(NCHUNK)]
    pair_base = [sum(NPU[:c]) for c in range(NCHUNK)]
    sing_base = [sum(NPU) + sum(NSE[:c]) for c in range(NCHUNK)]

    with tile.TileContext(nc) as tc, ExitStack() as ctx:
        idxp = ctx.enter_context(tc.tile_pool(name="idx", bufs=1))
        constp = ctx.enter_context(tc.tile_pool(name="const", bufs=1))
        zerop = ctx.enter_context(tc.tile_pool(name="zero", bufs=1))
        edgep = ctx.enter_context(tc.tile_pool(name="edge", bufs=4))
        grpp = ctx.enter_context(tc.tile_pool(name="grp", bufs=3))
        psA = ctx.enter_context(tc.tile_pool(name="psA", bufs=2, space="PSUM"))
        psB = ctx.enter_context(tc.tile_pool(name="psB", bufs=2, space="PSUM"))
        psC = ctx.enter_context(tc.tile_pool(name="psC", bufs=2, space="PSUM"))

        nc.gpsimd.load_library(library_config.mlp)

        # ---- persistent constants ----
        gi = idxp.tile([128, LG], i16)
        si = idxp.tile([128, LS], i16)
        nc.sync.dma_start(gi[:], gidx_p[:])
        nc.sync.dma_start(si[:], sidx_p[:])

        inv = constp.tile([128, NBLK], f32)
        nc.sync.dma_start(inv[:], inv_p[:])
        ident = constp.tile([128, 128], f32)
        masks.make_identity(nc, ident[:])
        ones = constp.tile([1, GROUP], f32)
        nc.vector.memset(ones[:], 1.0)

        wl_t, wr_t, b_t = [None], [None], [None]
        for l in range(1, 5):
            din = FIN if l == 1 else F
            dout = FOUT if l == 4 else F
            t1 = constp.tile([din, dout], f32, tag=f"wl{l}")
            t2 = constp.tile([din, dout], f32, tag=f"wr{l}")
            t3 = constp.tile([1, dout], f32, tag=f"b{l}")
            nc.sync.dma_start(t1[:], wl_p[l][:])
            nc.sync.dma_start(t2[:], wr_p[l][:])
            nc.sync.dma_start(t3[:], b_p[l][:])
            wl_t.append(t1)
            wr_t.append(t2)
            b_t.append(t3)

        zt = zerop.tile([128, (AGG_R // 128) * F], f32)
        nc.vector.memset(zt[:], 0.0)

        _REPS = int(os.environ.get("K_REPS", "1"))
        for _rep in range(_REPS):
            # ---- layer-1 table: T1 = x @ Wl1, node-major, then allgather ----
            for g0, nb in groups:
                xt = grpp.tile([FIN, GROUP], f32, tag="prevT")
                nc.sync.dma_start(
                    xt[:, : nb * 128], xT_p[:, g0 * 128 : (g0 + nb) * 128]
                )
                pnm = psC.tile([128, GROUP // 128, F], f32, tag="nm")
                for a in range(nb):
                    nc.tensor.matmul(
                        pnm[:, a, :],
                        xt[:, a * 128 : (a + 1) * 128],
                        wl_t[1][:],
                        start=True,
                        stop=True,
                    )
                hnm = grpp.tile([128, GROUP // 128, F], f32, tag="hnm")
                nc.scalar.activation(hnm[:, :nb, :], pnm[:, :nb, :], AF.Copy)
                nc.sync.dma_start(
                    sh[1][g0 * 128 : (g0 + nb) * 128, :].rearrange(
                        "(a p) f -> p a f", p=128
                    ),
                    hnm[:, :nb, :],
                )
            if not _SKIP_CC:
                nc.gpsimd.collective_compute(
                    "AllGather",
                    mybir.AluOpType.bypass,
                    replica_groups=[list(range(NCORES))],
                    ins=[sh[1][:]],
                    outs=[T[1][:]],
                )

            # ---- layers ----
            for l in range(1, 5):
                din = FIN if l == 1 else F
                dout = FOUT if l == 4 else F
                prev_dram = xT_p if l == 1 else hTd[l % 2]
                next_hT = hTd[(l + 1) % 2]

                # zero the 4 chunk accumulators
                for c in range(NCHUNK):
                    nc.sync.dma_start(
                        agg[l - 1][c].rearrange("(p a) f -> p (a f)", p=128), zt[:]
                    )

                # edge phase: gather + paired/single wave scatters, chunks
                # interleaved so the four wave-chains overlap
                percall = [chunk_calls(c) for c in range(NCHUNK)]
                maxcalls = max(len(p) for p in percall)
                for i in range(maxcalls):
                    for c in range(NCHUNK):
                        if _SKIP_EDGE or i >= len(percall[c]):
                            continue
                        off, n, pslices, sslices = percall[c][i]
                        tbl = T[l][c * CH : (c + 1) * CH, :]
                        gbase = chunk_base[c] + off
                        gt = edgep.tile([128, SUB // 128, F], f32, tag=f"e{c}")
                        nc.gpsimd.dma_gather(
                            gt[:, : n // 128, :],
                            tbl,
                            gi[:, gbase // 16 : (gbase + n) // 16],
                            n,
                            n,
                            F,
                            queue_num=c,
                        )
                        if _SCATTER_OFF:
                            continue
                        gtp = gt[:].rearrange(
                            "p (a two) f -> p a (two f)", two=2
                        )
                        aggp = agg[l - 1][c].rearrange(
                            "(a two) f -> a (two f)", two=2
                        )
                        for rel, cnt, au in pslices:
                            sb = pair_base[c] + au
                            nc.gpsimd.dma_scatter_add(
                                aggp,
                                gtp[:, rel // 128 : (rel + cnt) // 128, :],
                                si[:, sb // 16 : (sb + cnt) // 16],
                                cnt,
                                cnt,
                                2 * F,
                                queue_num=c,
                            )
                        for rel, cnt, ae in sslices:
                            sb = sing_base[c] + ae
                            nc.gpsimd.dma_scatter_add(
                                agg[l - 1][c][:],
                                gt[:, rel // 128 : (rel + cnt) // 128, :],
                                si[:, sb // 16 : (sb + cnt) // 16],
                                cnt,
                                cnt,
                                F,
                                queue_num=c,
                            )

                # epilogue per 512-node group
                for gidx_g, (g0, nb) in enumerate(groups):
                    if _SKIP_EPI and l < 4:
                        continue
                    rows = slice(g0 * 128, (g0 + nb) * 128)
                    asb = grpp.tile([128, GROUP // 128, F], f32, tag="agg")
                    nc.sync.dma_start(
                        asb[:, :nb, :],
                        agg[l - 1][0][rows, :].rearrange("(a p) f -> p a f", p=128),
                    )
                    for c in range(1, NCHUNK):
                        a2 = grpp.tile([128, GROUP // 128, F], f32, tag=f"agg{c}")
                        nc.sync.dma_start(
                            a2[:, :nb, :],
                            agg[l - 1][c][rows, :].rearrange("(a p) f -> p a f", p=128),
                        )
                        nc.vector.tensor_tensor(
                            asb[:, :nb, :], asb[:, :nb, :], a2[:, :nb, :], ALU.add
                        )
                    # mean scale (per-partition scalar per block)
                    for a in range(nb):
                        nc.vector.tensor_scalar(
                            asb[:, a, :],
                            asb[:, a, :],
                            inv[:, g0 + a : g0 + a + 1],
                            None,
                            ALU.mult,
                        )
                    # transpose to feature-major
                    pmt = psA.tile([F, GROUP], f32, tag="mt")
                    for a in range(nb):
                        nc.tensor.transpose(
                            pmt[:, a * 128 : (a + 1) * 128], asb[:, a, :], ident[:]
                        )
                    mt = grpp.tile([F, GROUP], f32, tag="mt_sb")
                    nc.vector.tensor_copy(mt[:, : nb * 128], pmt[:, : nb * 128])

                    # prev features (feature-major) for the Wr part
                    pv = grpp.tile([din, GROUP], f32, tag="prevT")
                    nc.sync.dma_start(pv[:, : nb * 128], prev_dram[:, rows])

                    ph = psB.tile([dout, GROUP], f32, tag="h")
                    if l == 1:
                        nc.tensor.matmul(
                            ph[:, : nb * 128],
                            ident[:F, :F],
                            mt[:, : nb * 128],
                            start=True,
                            stop=False,
                        )
                    else:
                        nc.tensor.matmul(
                            ph[:, : nb * 128],
                            wl_t[l][:],
                            mt[:, : nb * 128],
                            start=True,
                            stop=False,
                        )
                    nc.tensor.matmul(
                        ph[:, : nb * 128],
                        wr_t[l][:],
                        pv[:, : nb * 128],
                        start=False,
                        stop=False,
                    )
                    nc.tensor.matmul(
                        ph[:, : nb * 128],
                        b_t[l][:],
                        ones[:, : nb * 128],
                        start=False,
                        stop=True,
                    )

                    if l < 4:
                        hT_sb = grpp.tile([F, GROUP], f32, tag="hT_sb")
                        nc.scalar.activation(
                            hT_sb[:, : nb * 128], ph[:, : nb * 128], AF.Relu
                        )
                        nc.sync.dma_start(next_hT[:, rows], hT_sb[:, : nb * 128])
                        # node-major for the next table
                        pnm = psC.tile([128, GROUP // 128, F], f32, tag="nm")
                        for a in range(nb):
                            nc.tensor.transpose(
                                pnm[:, a, :],
                                hT_sb[:, a * 128 : (a + 1) * 128],
                                ident[:F, :F],
                            )
                        hnm = grpp.tile([128, GROUP // 128, F], f32, tag="hnm")
                        nc.vector.tensor_copy(hnm[:, :nb, :], pnm[:, :nb, :])
                        nc.sync.dma_start(
                            sh[l + 1][rows, :].rearrange("(a p) f -> p a f", p=128),
                            hnm[:, :nb, :],
                        )
                    else:
                        # logits -> node-major -> log_softmax -> out_shard
                        zsb = grpp.tile([FOUT, GROUP], f32, tag="zsb")
                        nc.vector.tensor_copy(zsb[:, : nb * 128], ph[:, : nb * 128])
                        pz = psC.tile([128, GROUP // 128, FOUT], f32, tag="znm")
                        for a in range(nb):
                            nc.tensor.transpose(
                                pz[:, a, :],
                                zsb[:, a * 128 : (a + 1) * 128],
                                ident[:FOUT, :FOUT],
                            )
                        z = grpp.tile([128, GROUP // 128, FOUT], f32, tag="z")
                        nc.vector.tensor_copy(z[:, :nb, :], pz[:, :nb, :])
                        z0 = z[:, :nb, 0:1]
                        z1 = z[:, :nb, 1:2]
                        m = grpp.tile([128, GROUP // 128, 1], f32, tag="m")
                        nc.vector.tensor_tensor(m[:, :nb, :], z0, z1, ALU.max)
                        d = grpp.tile([128, GROUP // 128, FOUT], f32, tag="d")
                        nc.vector.tensor_tensor(d[:, :nb, 0:1], z0, m[:, :nb, :], ALU.subtract)
                        nc.vector.tensor_tensor(d[:, :nb, 1:2], z1, m[:, :nb, :], ALU.subtract)
                        e = grpp.tile([128, GROUP // 128, FOUT], f32, tag="e")
                        nc.scalar.activation(e[:, :nb, :], d[:, :nb, :], AF.Exp)
                        s = grpp.tile([128, GROUP // 128, 1], f32, tag="s")
                        nc.vector.tensor_tensor(
                            s[:, :nb, :], e[:, :nb, 0:1], e[:, :nb, 1:2], ALU.add
                        )
                        ls = grpp.tile([128, GROUP // 128, 1], f32, tag="ls")
                        nc.scalar.activation(ls[:, :nb, :], s[:, :nb, :], AF.Ln)
                        o = grpp.tile([128, GROUP // 128, FOUT], f32, tag="o")
                        nc.vector.tensor_tensor(
                            o[:, :nb, 0:1], d[:, :nb, 0:1], ls[:, :nb, :], ALU.subtract
                        )
                        nc.vector.tensor_tensor(
                            o[:, :nb, 1:2], d[:, :nb, 1:2], ls[:, :nb, :], ALU.subtract
                        )
                        nc.sync.dma_start(
                            out_p[rows, :].rearrange("(a p) f -> p a f", p=128),
                            o[:, :nb, :],
                        )

                if l < 4 and not (_SKIP_CC or _SKIP_EPI):
                    nc.gpsimd.collective_compute(
                        "AllGather",
                        mybir.AluOpType.bypass,
                        replica_groups=[list(range(NCORES))],
                        ins=[sh[l + 1][:]],
                        outs=[T[l + 1][:]],
                    )

    nc.compile()
    return nc


def _prepare(inputs):
    x = np.asarray(inputs["x"], dtype=np.float32)
    edge_index = np.asarray(inputs["edge_index"])
    meta, gidx_maps, sidx_maps, xT, inv_pb = _preprocess(x, edge_index)

    key = (
        tuple(meta["P"]),
        tuple(meta["NPU"]),
        tuple(meta["NSE"]),
        _SKIP_EDGE,
        _SKIP_CC,
        _SKIP_EPI,
        _SCATTER_OFF,
        os.environ.get("K_REPS", "1"),
    )
    if key not in _CACHE:
        _CACHE[key] = _build_module(meta)
    nc = _CACHE[key]

    in_maps = []
    for k in range(NCORES):
        m = {
            "xT": xT[k],
            "gidx": gidx_maps[k],
            "sidx": sidx_maps[k],
            "invdeg": inv_pb[k],
        }
        for l in range(1, 5):
            m[f"Wl{l}"] = np.asarray(inputs[f"Wl{l}"], np.float32)
            m[f"Wr{l}"] = np.asarray(inputs[f"Wr{l}"], np.float32)
            m[f"b{l}"] = np.asarray(inputs[f"b{l}"], np.float32).reshape(1, -1)
        in_maps.append(m)
    return nc, in_maps


def _run(inputs, trace=False):
    from concourse.bass_utils import run_bass_kernel_spmd

    nc, in_maps = _prepare(inputs)
    r = run_bass_kernel_spmd(nc, in_maps, list(range(NCORES)), trace=trace)
    out = np.concatenate(
        [r.results[k]["out_shard"] for k in range(NCORES)], axis=0
    )[:N]
    return out.astype(np.float32), r


def kernel(**inputs) -> np.ndarray:
    out, _ = _run(inputs)
    return out



# revision 22
# speedup vs baseline: 3.4216x; 2.1061x over previous
"""Bass/Trainium2 kernel for a 4-layer GraphSAGE GNN (mean aggregation).

Problem (hardcoded): N=100000 nodes, E=1200000 edges, x:[N,3] f32,
edge_index:[2,E] int64, hidden=64, out=2, log_softmax output.

  h1 = relu(mean_nbr(x) @ Wl1 + x @ Wr1 + b1)
  h2 = relu(mean_nbr(h1) @ Wl2 + h1 @ Wr2 + b2)
  h3 = relu(mean_nbr(h2) @ Wl3 + h2 @ Wr3 + b3)
  out = log_softmax(mean_nbr(h3) @ Wl4 + h3 @ Wr4 + b4)

Strategy (8 NeuronCores, node-partitioned, v2 = gather + PE aggregation):
- Core k owns nodes [k*12544, (k+1)*12544). Per layer each core gathers its
  in-edge source rows (256B) from the allgathered node table with
  dma_gather, 4 streams keyed by src chunk (table quarter, so indices fit
  int16), one SWDGE queue per stream for ring-level parallelism.
- No scatter: aggregation is matmul. Edges are pre-sorted by dst tile
  (128 dst nodes); for each tile a PSUM tile [64 feat, 128 dst]
  accumulates  G_g.T @ Sw_g  over the tile's edge groups, where G_g is
  the gathered bf16 feature block (partitions = edges) and Sw_g is a
  host-built bf16 selection matrix with 1/deg folded in. The result IS
  the feature-major mean, so the epilogue needs no transpose and no
  div-by-degree.
- Epilogue per 512 nodes: 3 PSUM matmuls (Wl/Wr/bias-rank-1), relu on
  ACT; h kept feature-major in a DRAM ping-pong for the Wr matmul and
  node-major in the allgathered table for the next layer's gather.
- Layer 1 aggregates T1 = x @ Wl1 (so gather rows are 256B even though
  x rows are only 12B); layer 1's mean-part matmul is an identity.
- Group structure (edge counts per (tile, chunk) run, padded to x64) is
  shared across cores (max over cores) so all 8 cores run one program;
  shorter cores pad with zero-weight edges.
"""

import os
import numpy as np
from contextlib import ExitStack

# debug/bench switches (unset in normal use)
_SKIP_EDGE = os.environ.get("K_SKIP_EDGE", "") != ""
_SKIP_MM = os.environ.get("K_SKIP_MM", "") != ""
_SKIP_CC = os.environ.get("K_SKIP_CC", "") != ""
_SKIP_EPI = os.environ.get("K_SKIP_EPI", "") != ""
_MM_CONST = os.environ.get("K_MM_CONST", "") != ""   # matmuls vs const rhs, no Sw dma
_SW_ONLY = os.environ.get("K_SW_ONLY", "") != ""     # Sw dmas only, no matmuls

# ---- problem constants (self-contained; do not read spec/reference) ----
N = 100000
E = 1200000
NCORES = 8
NPC = -(-N // (NCORES * 128)) * 128  # nodes per core = 12544 = 98 * 128
NPAD = NCORES * NPC         # 100352
NCHUNK = 4
CH = NPAD // NCHUNK         # 25088 table rows per chunk (int16-safe)
F = 64
FIN = 3
FOUT = 2
SUB = int(os.environ.get("K_SUB", "1024"))  # edges per gather call
SCRATCH = 16384 if SUB <= 1024 else 32 * SUB  # SWDGE ring carveout (2 calls/queue)
GROUP = 512                 # nodes per epilogue group
NBLK = NPC // 128           # tiles per core = 98
GRAN = int(os.environ.get("K_GRAN", "64"))  # run padding granularity (64|128)

_CACHE = {}


def _wrap_idx(idx: np.ndarray) -> np.ndarray:
    """Edge i -> idxs[i%16, i//16], replicated for the 8 Q7 cores."""
    w = idx.reshape(-1, 16).T.astype(np.int16)
    return np.tile(w, (8, 1))


def _group_schedule(pad_tc):
    """Shared group schedule from the padded (tile, chunk) run lengths.

    Returns (groups, ncalls) where groups[t] is a list of
    (c, call_idx, slice_idx, p0, gsz, colbase) in accumulation order and
    colbase assigns each group 128 Sw columns, tile-major.
    """
    nt = pad_tc.shape[0]
    off = [0] * NCHUNK
    groups = [[] for _ in range(nt)]
    run_off = np.zeros((nt, NCHUNK), np.int64)
    colbase = 0
    for t in range(nt):
        for c in range(NCHUNK):
            run_off[t, c] = off[c]
            rem = int(pad_tc[t, c])
            while rem > 0:
                o = off[c]
                # runs start x128-aligned, so p0 is always 0 (the PE rejects
                # partition-offset-64 operands on HW)
                assert o % 128 == 0
                gsz = 128 if rem >= 128 else 64
                groups[t].append((c, o // SUB, (o % SUB) // 128, 0, gsz, colbase))
                colbase += 128
                off[c] += gsz
                rem -= gsz
            off[c] = -(-off[c] // 128) * 128
    lens = [int(-(-off[c] // SUB)) for c in range(NCHUNK)]
    return groups, lens, run_off, colbase


def _preprocess(x: np.ndarray, edge_index: np.ndarray):
    import ml_dtypes

    src = np.asarray(edge_index[0], dtype=np.int64)
    dst = np.asarray(edge_index[1], dtype=np.int64)

    deg = np.bincount(dst, minlength=NPAD).astype(np.float32)
    invdeg = (1.0 / np.maximum(deg, 1.0)).astype(np.float32)

    owner = dst // NPC
    # per-core sorted edge structure
    per_core = []
    counts = np.zeros((NCORES, NBLK, NCHUNK), np.int64)
    for k in range(NCORES):
        m = owner == k
        s_k = src[m]
        d_k = dst[m] - k * NPC
        t_k = d_k >> 7
        c_k = s_k // CH
        key = t_k * NCHUNK + c_k
        order = np.argsort(key, kind="stable")
        s_k, d_k, t_k, c_k, key = (
            s_k[order], d_k[order], t_k[order], c_k[order], key[order]
        )
        cnt = np.bincount(key, minlength=NBLK * NCHUNK).reshape(NBLK, NCHUNK)
        counts[k] = cnt
        per_core.append((s_k, d_k, c_k, key))

    maxc = counts.max(axis=0)
    pad_tc = -(-maxc // GRAN) * GRAN  # ceil to run granularity
    # every tile needs >= 1 group so its PSUM gets start/stop
    empty = pad_tc.sum(axis=1) == 0
    pad_tc[empty, 0] = GRAN

    groups, ncalls, run_off, swcols = _group_schedule(pad_tc)
    # equalize call counts across chunks: the module emits gather calls in
    # strict 0,1,2,3 queue round-robin so the tile DMASW semaphore lanes
    # (8, assigned round-robin) each see a single SWDGE queue.
    ncalls = [max(ncalls)] * NCHUNK
    L = [ncalls[c] * SUB for c in range(NCHUNK)]

    # group col lookup: for (t, c) run, the Sw column base of each x64 unit
    # unit u of run (t,c) belongs to group index: rebuild per run from groups
    gidx_maps, sw_maps = [], []
    for k in range(NCORES):
        s_k, d_k, c_k, key = per_core[k]
        cnt = counts[k]
        # position of each edge within its (t,c) run
        run_start_of_key = np.zeros(NBLK * NCHUNK, np.int64)
        np.cumsum(cnt.reshape(-1)[:-1], out=run_start_of_key[1:])
        rank = np.arange(len(s_k)) - run_start_of_key[key]
        # stream position = run_off[t,c] + rank
        pos = run_off[d_k >> 7, c_k] + rank

        # gather index streams
        streams = []
        for c in range(NCHUNK):
            st = np.zeros(L[c], np.int64)
            mc = c_k == c
            st[pos[mc]] = s_k[mc] - c * CH
            streams.append(st)
        gidx_maps.append(_wrap_idx(np.concatenate(streams)))

        # Sw: [128, swcols] f32 -> bf16; entry at [pos%128, colbase+slot]
        sw = np.zeros((128, swcols), np.float32)
        # per-edge column base: group of unit (rank//64) of run (t,c)
        # build unit->colbase map per (t,c)
        unit_cb = np.zeros((NBLK, NCHUNK, int(pad_tc.max()) // 64), np.int64)
        for t in range(NBLK):
            off_c = {c: 0 for c in range(NCHUNK)}
            for (c, ci, sl, p0, gsz, cb) in groups[t]:
                u0 = off_c[c] // 64
                for u in range(gsz // 64):
                    unit_cb[t, c, u0 + u] = cb
                off_c[c] += gsz
        tt = d_k >> 7
        cb_e = unit_cb[tt, c_k, rank // 64]
        slot = d_k & 127
        sw[pos % 128, cb_e + slot] = invdeg[k * NPC + d_k]
        sw_maps.append(sw.astype(ml_dtypes.bfloat16))

    # per-core transposed features [fin, NPC]
    xpad = np.zeros((NPAD, FIN), np.float32)
    xpad[:N] = x
    xT = [
        np.ascontiguousarray(xpad[k * NPC : (k + 1) * NPC].T) for k in range(NCORES)
    ]
    meta = dict(
        pad_tc=pad_tc, groups=groups, ncalls=ncalls, swcols=swcols, L=L
    )
    return meta, gidx_maps, sw_maps, xT


def _build_module(meta):
    import concourse.bass as bass
    import concourse.bacc as bacc
    import concourse.mybir as mybir
    from concourse import tile
    from concourse import library_config
    from concourse import masks

    f32 = mybir.dt.float32
    bf16 = mybir.dt.bfloat16
    i16 = mybir.dt.int16
    AF = mybir.ActivationFunctionType
    ALU = mybir.AluOpType

    groups, ncalls, swcols = meta["groups"], meta["ncalls"], meta["swcols"]
    L = meta["L"]
    LG = sum(L) // 16
    stream_base = [sum(L[:c]) for c in range(NCHUNK)]
    nc = bacc.Bacc(
        None,
        target_bir_lowering=False,
        num_swdge_queues=4,
        dynamic_dma_scratch_size=SCRATCH,
    )

    # ---- parameters ----
    xT_p = nc.declare_dram_parameter("xT", [FIN, NPC], f32, isOutput=False)
    gidx_p = nc.declare_dram_parameter("gidx", [128, LG], i16, isOutput=False)
    sw_p = nc.declare_dram_parameter("Sw", [128, swcols], bf16, isOutput=False)
    wl_p, wr_p, b_p = [None], [None], [None]
    for l in range(1, 5):
        din = FIN if l == 1 else F
        dout = FOUT if l == 4 else F
        wl_p.append(nc.declare_dram_parameter(f"Wl{l}", [din, dout], f32, isOutput=False))
        wr_p.append(nc.declare_dram_parameter(f"Wr{l}", [din, dout], f32, isOutput=False))
        b_p.append(nc.declare_dram_parameter(f"b{l}", [1, dout], f32, isOutput=False))
    out_p = nc.declare_dram_parameter("out_shard", [NPC, FOUT], f32, isOutput=True)

    # ---- internal DRAM ----
    T = [None] + [
        nc.dram_tensor(f"T{l}", [NPAD, F], f32, addr_space="Shared") for l in range(1, 5)
    ]
    sh = [None] + [nc.dram_tensor(f"sh{l}", [NPC, F], f32) for l in range(1, 5)]
    hTd = [nc.dram_tensor(f"hT{i}", [F, NPC], f32) for i in range(2)]  # ping-pong

    # epilogue groups: (start_block, n_blocks)
    egroups = []
    b0 = 0
    while b0 < NBLK:
        nb = min(GROUP // 128, NBLK - b0)
        egroups.append((b0, nb))
        b0 += nb

    with tile.TileContext(nc) as tc, ExitStack() as ctx:
        idxp = ctx.enter_context(tc.tile_pool(name="idx", bufs=1))
        constp = ctx.enter_context(tc.tile_pool(name="const", bufs=1))
        gtp = [
            ctx.enter_context(tc.tile_pool(name=f"gt{c}", bufs=2))
            for c in range(NCHUNK)
        ]
        gbp = [
            ctx.enter_context(tc.tile_pool(name=f"gb{c}", bufs=3))
            for c in range(NCHUNK)
        ]
        swp = ctx.enter_context(tc.tile_pool(name="sw", bufs=3))
        grpp = ctx.enter_context(tc.tile_pool(name="grp", bufs=3))
        psA = ctx.enter_context(tc.tile_pool(name="psA", bufs=2, space="PSUM"))
        psB = ctx.enter_context(tc.tile_pool(name="psB", bufs=2, space="PSUM"))
        psC = ctx.enter_context(tc.tile_pool(name="psC", bufs=2, space="PSUM"))

        nc.gpsimd.load_library(library_config.mlp)

        # ---- persistent constants ----
        gi = idxp.tile([128, LG], i16)
        nc.sync.dma_start(gi[:], gidx_p[:])

        ident = constp.tile([128, 128], f32)
        masks.make_identity(nc, ident[:])
        ones = constp.tile([1, GROUP], f32)
        nc.vector.memset(ones[:], 1.0)
        swconst = None
        if _MM_CONST:
            swconst = constp.tile([128, 128], bf16, tag="swconst")
            nc.vector.memset(swconst[:], 0.0)

        wl_t, wr_t, b_t = [None], [None], [None]
        for l in range(1, 5):
            din = FIN if l == 1 else F
            dout = FOUT if l == 4 else F
            t1 = constp.tile([din, dout], f32, tag=f"wl{l}")
            t2 = constp.tile([din, dout], f32, tag=f"wr{l}")
            t3 = constp.tile([1, dout], f32, tag=f"b{l}")
            nc.sync.dma_start(t1[:], wl_p[l][:])
            nc.sync.dma_start(t2[:], wr_p[l][:])
            nc.sync.dma_start(t3[:], b_p[l][:])
            wl_t.append(t1)
            wr_t.append(t2)
            b_t.append(t3)

        _REPS = int(os.environ.get("K_REPS", "1"))
        for _rep in range(_REPS):
            # ---- layer-1 table: T1 = x @ Wl1, node-major, then allgather ----
            for g0, nb in egroups:
                xt = grpp.tile([FIN, GROUP], f32, tag="prevT")
                nc.sync.dma_start(
                    xt[:, : nb * 128], xT_p[:, g0 * 128 : (g0 + nb) * 128]
                )
                pnm = psC.tile([128, GROUP // 128, F], f32, tag="nm")
                for a in range(nb):
                    nc.tensor.matmul(
                        pnm[:, a, :],
                        xt[:, a * 128 : (a + 1) * 128],
                        wl_t[1][:],
                        start=True,
                        stop=True,
                    )
                hnm = grpp.tile([128, GROUP // 128, F], f32, tag="hnm")
                nc.scalar.activation(hnm[:, :nb, :], pnm[:, :nb, :], AF.Copy)
                nc.sync.dma_start(
                    sh[1][g0 * 128 : (g0 + nb) * 128, :].rearrange(
                        "(a p) f -> p a f", p=128
                    ),
                    hnm[:, :nb, :],
                )
            if not _SKIP_CC:
                nc.gpsimd.collective_compute(
                    "AllGather",
                    mybir.AluOpType.bypass,
                    replica_groups=[list(range(NCORES))],
                    ins=[sh[1][:]],
                    outs=[T[1][:]],
                )

            # ---- layers ----
            for l in range(1, 5):
                din = FIN if l == 1 else F
                dout = FOUT if l == 4 else F
                prev_dram = xT_p if l == 1 else hTd[l % 2]
                next_hT = hTd[(l + 1) % 2]

                # gather + bf16 convert, 4 chunk streams on 4 queues
                gb_tiles = [[None] * ncalls[c] for c in range(NCHUNK)]
                maxcalls = max(ncalls)
                for i in range(maxcalls):
                    for c in range(NCHUNK):
                        if _SKIP_EDGE or i >= ncalls[c]:
                            continue
                        tbl = T[l][c * CH : (c + 1) * CH, :]
                        gbase = stream_base[c] + i * SUB
                        gt = gtp[c].tile([128, SUB // 128, F], f32, tag=f"e{c}")
                        nc.gpsimd.dma_gather(
                            gt[:],
                            tbl,
                            gi[:, gbase // 16 : (gbase + SUB) // 16],
                            SUB,
                            SUB,
                            F,
                            queue_num=c,
                        )
                        gb = gbp[c].tile([128, SUB // 128, F], bf16, tag=f"b{c}")
                        nc.scalar.activation(gb[:], gt[:], AF.Copy)
                        gb_tiles[c][i] = gb

                # aggregation matmuls + epilogue per 512-node group
                for g0, nb in egroups:
                    if _SKIP_EPI and l < 4:
                        continue
                    rows = slice(g0 * 128, (g0 + nb) * 128)
                    mt = grpp.tile([F, GROUP], f32, tag="mt")
                    for a in range(nb):
                        t = g0 + a
                        pmt = psA.tile([F, 128], f32, tag="agg")
                        if _SKIP_EDGE or _SKIP_MM:
                            nc.vector.memset(mt[:, a * 128 : (a + 1) * 128], 0.0)
                            continue
                        # per-tile Sw columns
                        cb0 = groups[t][0][5]
                        cb1 = groups[t][-1][5] + 128
                        if not _MM_CONST:
                            swt = swp.tile([128, cb1 - cb0], bf16, tag="sw")
                            nc.sync.dma_start(swt[:], sw_p[:, cb0:cb1])
                        if _SW_ONLY:
                            nc.vector.memset(mt[:, a * 128 : (a + 1) * 128], 0.0)
                            continue
                        ng = len(groups[t])
                        for j, (c, ci, sl, p0, gsz, cb) in enumerate(groups[t]):
                            gb = gb_tiles[c][ci]
                            rhs = (
                                swconst[p0 : p0 + gsz, :]
                                if _MM_CONST
                                else swt[p0 : p0 + gsz, cb - cb0 : cb - cb0 + 128]
                            )
                            nc.tensor.matmul(
                                pmt[:],
                                gb[p0 : p0 + gsz, sl, :],
                                rhs,
                                start=(j == 0),
                                stop=(j == ng - 1),
                            )
                        nc.scalar.activation(
                            mt[:, a * 128 : (a + 1) * 128], pmt[:], AF.Copy
                        )

                    # prev features (feature-major) for the Wr part
                    pv = grpp.tile([din, GROUP], f32, tag="prevT")
                    nc.sync.dma_start(pv[:, : nb * 128], prev_dram[:, rows])

                    ph = psB.tile([dout, GROUP], f32, tag="h")
                    if l == 1:
                        nc.tensor.matmul(
                            ph[:, : nb * 128],
                            ident[:F, :F],
                            mt[:, : nb * 128],
                            start=True,
                            stop=False,
                        )
                    else:
                        nc.tensor.matmul(
                            ph[:, : nb * 128],
                            wl_t[l][:],
                            mt[:, : nb * 128],
                            start=True,
                            stop=False,
                        )
                    nc.tensor.matmul(
                        ph[:, : nb * 128],
                        wr_t[l][:],
                        pv[:, : nb * 128],
                        start=False,
                        stop=False,
                    )
                    nc.tensor.matmul(
                        ph[:, : nb * 128],
                        b_t[l][:],
                        ones[:, : nb * 128],
                        start=False,
                        stop=True,
                    )

                    if l < 4:
                        hT_sb = grpp.tile([F, GROUP], f32, tag="hT_sb")
                        nc.scalar.activation(
                            hT_sb[:, : nb * 128], ph[:, : nb * 128], AF.Relu
                        )
                        nc.sync.dma_start(next_hT[:, rows], hT_sb[:, : nb * 128])
                        # node-major for the next table
                        pnm = psC.tile([128, GROUP // 128, F], f32, tag="nm")
                        for a in range(nb):
                            nc.tensor.transpose(
                                pnm[:, a, :],
                                hT_sb[:, a * 128 : (a + 1) * 128],
                                ident[:F, :F],
                            )
                        hnm = grpp.tile([128, GROUP // 128, F], f32, tag="hnm")
                        nc.vector.tensor_copy(hnm[:, :nb, :], pnm[:, :nb, :])
                        nc.sync.dma_start(
                            sh[l + 1][rows, :].rearrange("(a p) f -> p a f", p=128),
                            hnm[:, :nb, :],
                        )
                    else:
                        # logits -> node-major -> log_softmax -> out_shard
                        zsb = grpp.tile([FOUT, GROUP], f32, tag="zsb")
                        nc.vector.tensor_copy(zsb[:, : nb * 128], ph[:, : nb * 128])
                        pz = psC.tile([128, GROUP // 128, FOUT], f32, tag="znm")
                        for a in range(nb):
                            nc.tensor.transpose(
                                pz[:, a, :],
                                zsb[:, a * 128 : (a + 1) * 128],
                                ident[:FOUT, :FOUT],
                            )
                        z = grpp.tile([128, GROUP // 128, FOUT], f32, tag="z")
                        nc.vector.tensor_copy(z[:, :nb, :], pz[:, :nb, :])
                        z0 = z[:, :nb, 0:1]
                        z1 = z[:, :nb, 1:2]
                        m = grpp.tile([128, GROUP // 128, 1], f32, tag="m")
                        nc.vector.tensor_tensor(m[:, :nb, :], z0, z1, ALU.max)
                        d = grpp.tile([128, GROUP // 128, FOUT], f32, tag="d")
                        nc.vector.tensor_tensor(d[:, :nb, 0:1], z0, m[:, :nb, :], ALU.subtract)
                        nc.vector.tensor_tensor(d[:, :nb, 1:2], z1, m[:, :nb, :], ALU.subtract)
                        e = grpp.tile([128, GROUP // 128, FOUT], f32, tag="e")
                        nc.scalar.activation(e[:, :nb, :], d[:, :nb, :], AF.Exp)
                        s = grpp.tile([128, GROUP // 128, 1], f32, tag="s")
                        nc.vector.tensor_tensor(
                            s[:, :nb, :], e[:, :nb, 0:1], e[:, :nb, 1:2], ALU.add
                        )
                        ls = grpp.tile([128, GROUP // 128, 1], f32, tag="ls")
                        nc.scalar.activation(ls[:, :nb, :], s[:, :nb, :], AF.Ln)
                        o = grpp.tile([128, GROUP // 128, FOUT], f32, tag="o")
                        nc.vector.tensor_tensor(
                            o[:, :nb, 0:1], d[:, :nb, 0:1], ls[:, :nb, :], ALU.subtract
                        )
                        nc.vector.tensor_tensor(
                            o[:, :nb, 1:2], d[:, :nb, 1:2], ls[:, :nb, :], ALU.subtract
                        )
                        nc.sync.dma_start(
                            out_p[rows, :].rearrange("(a p) f -> p a f", p=128),
                            o[:, :nb, :],
                        )

                if l < 4 and not (_SKIP_CC or _SKIP_EPI):
                    nc.gpsimd.collective_compute(
                        "AllGather",
                        mybir.AluOpType.bypass,
                        replica_groups=[list(range(NCORES))],
                        ins=[sh[l + 1][:]],
                        outs=[T[l + 1][:]],
                    )

    nc.compile()
    return nc


def _prepare(inputs):
    x = np.asarray(inputs["x"], dtype=np.float32)
    edge_index = np.asarray(inputs["edge_index"])
    meta, gidx_maps, sw_maps, xT = _preprocess(x, edge_index)

    key = (
        meta["pad_tc"].tobytes(),
        _SKIP_EDGE,
        _SKIP_MM,
        _SKIP_CC,
        _SKIP_EPI,
        _MM_CONST,
        _SW_ONLY,
        GRAN,
        SUB,
        os.environ.get("K_REPS", "1"),
    )
    if key not in _CACHE:
        _CACHE[key] = _build_module(meta)
    nc = _CACHE[key]

    in_maps = []
    for k in range(NCORES):
        m = {
            "xT": xT[k],
            "gidx": gidx_maps[k],
            "Sw": sw_maps[k],
        }
        for l in range(1, 5):
            m[f"Wl{l}"] = np.asarray(inputs[f"Wl{l}"], np.float32)
            m[f"Wr{l}"] = np.asarray(inputs[f"Wr{l}"], np.float32)
            m[f"b{l}"] = np.asarray(inputs[f"b{l}"], np.float32).reshape(1, -1)
        in_maps.append(m)
    return nc, in_maps


def _run(inputs, trace=False):
    from concourse.bass_utils import run_bass_kernel_spmd

    nc, in_maps = _prepare(inputs)
    r = run_bass_kernel_spmd(nc, in_maps, list(range(NCORES)), trace=trace)
    out = np.concatenate(
        [r.results[k]["out_shard"] for k in range(NCORES)], axis=0
    )[:N]
    return out.astype(np.float32), r


def kernel(**inputs) -> np.ndarray:
    out, _ = _run(inputs)
    return out
